# revision 1
# baseline (speedup 1.0000x reference)
"""ContrastiveLoss Trainium2 kernel.

Contract: kernel(feature, label) -> (loss, mean_pos, mean_neg), matching
reference.reference(). Full inputs in, full outputs out; internally sharded
across 8 NeuronCores.

Strategy: rows sorted by label on host (label-1 block first, n1 rows), so the
label-weighted exp row sums come free as range-split accum_out from the
scalar-engine exp pass. Each core receives the full sorted feature (columns)
plus its own 1024-row slice (rows) -> one uniform SPMD program, no
collectives; host sums the per-core scalar partials.
"""

import sys

sys.path.insert(0, "/opt/trn_rl_repo")

import numpy as np

import concourse.bass as bass
import concourse.mybir as mybir
import concourse.tile as tile
from concourse import bacc
from concourse.bass import ds, ts
from concourse.bass_utils import run_bass_kernel_spmd
from concourse.masks import make_identity

B = 8192
D = 128
N_CORES = 8
LOCAL = B // N_CORES          # 1024 rows per core
TILES_I = LOCAL // 128        # 8 local row tiles per core
GROUPS = B // LOCAL           # 8 fsort groups of 8 tiles
CHUNK = 2048                  # psum chunk width (4 banks)
NCHUNK = B // CHUNK
SUB = 512                     # matmul moving width & zTrc tile width
NSUB = B // SUB               # 16 column tiles
EPS = 1e-8

FP32 = mybir.dt.float32
F32R = mybir.dt.float32r
AF = mybir.ActivationFunctionType
ALU = mybir.AluOpType
AX = mybir.AxisListType


def _build_kernel(n1: int):
    nc = bacc.Bacc("TRN2", target_bir_lowering=False, debug=False,
                   num_devices=N_CORES)
    fsort = nc.dram_tensor("fsort", [B, D], FP32, kind="ExternalInput").ap()
    floc = nc.dram_tensor("floc", [LOCAL, D], FP32, kind="ExternalInput").ap()
    labloc = nc.dram_tensor("labloc", [128, TILES_I], FP32,
                            kind="ExternalInput").ap()
    outp = nc.dram_tensor("outp", [128, 3 * TILES_I], FP32,
                          kind="ExternalOutput").ap()

    # activation segments per row tile: (lo, hi, is_label1), label-1 first
    segs = []
    for q in range(NCHUNK):
        lo, hi = q * CHUNK, (q + 1) * CHUNK
        if n1 <= lo:
            segs.append((lo, hi, False))
        elif n1 >= hi:
            segs.append((lo, hi, True))
        else:
            segs.append((lo, n1, True))
            segs.append((n1, hi, False))
    segs = [s for s in segs if s[2]] + [s for s in segs if not s[2]]
    nseg = len(segs)
    cb = sum(1 for s in segs if s[2])

    with tile.TileContext(nc) as tc:
        with (
            tc.tile_pool(name="zc", bufs=1) as zc,
            tc.tile_pool(name="fgp", bufs=1) as fgp,
            tc.tile_pool(name="deadp", bufs=2) as deadp,
            tc.tile_pool(name="small", bufs=1) as small,
            tc.tile_pool(name="psum", bufs=2, space=bass.MemorySpace.PSUM) as psum,
        ):
            ztc = [zc.tile([128, SUB], F32R, tag=f"ztc{k}", name=f"ztc{k}")
                   for k in range(NSUB)]
            zlT = zc.tile([128, LOCAL], F32R, tag="zlT")
            ident = small.tile([128, 128], FP32, tag="ident")
            make_identity(nc, ident[:])

            lab_loc = small.tile([128, TILES_I], FP32, tag="labloc")
            nc.gpsimd.dma_start(lab_loc[:], labloc)

            nrm2l = small.tile([128, TILES_I], FP32, tag="nrm2l")
            scll = small.tile([128, TILES_I], FP32, tag="scll")

            def norm_group(src3, fg, nrm2c, sclc, dma_eng):
                """Load one 8-tile group, compute row norms, scale rows."""
                dma_eng.dma_start(fg[:], src3)
                for t in range(TILES_I):
                    dead = deadp.tile([128, 128], FP32, tag="dead")
                    nc.vector.scalar_tensor_tensor(
                        out=dead[:], in0=fg[:, t, :], scalar=1.0,
                        in1=fg[:, t, :], op0=ALU.mult, op1=ALU.mult,
                        accum_out=nrm2c[:, t:t + 1])
                # scl = exp(-0.5*ln(max(nrm2,1e-24))) = 1/max(||f||,1e-12)
                nc.vector.tensor_scalar_max(out=sclc[:], in0=nrm2c[:],
                                            scalar1=1e-24)
                nc.scalar.activation(sclc[:], sclc[:], AF.Ln)
                nc.scalar.activation(sclc[:], sclc[:], AF.Exp, scale=-0.5)
                for t in range(TILES_I):
                    nc.gpsimd.tensor_scalar_mul(
                        out=fg[:, t, :], in0=fg[:, t, :],
                        scalar1=sclc[:, t:t + 1])

            def transpose_group(fg, dst):
                """dst[h] <- transpose of fg tiles 4h..4h+3 ([128,512] each)."""
                for h in range(2):
                    pt = psum.tile([128, CHUNK], FP32, tag="ps")
                    for k in range(4):
                        nc.tensor.transpose(pt[:, ts(k, 128)],
                                            fg[:, h * 4 + k, :], ident[:])
                    nc.vector.tensor_copy(dst[h], pt[:, 0:SUB])

            # local rows first (zlT needed by every main matmul)
            fl = fgp.tile([128, TILES_I, D], FP32, tag="fl")
            norm_group(floc.rearrange("(t p) d -> p t d", p=128), fl,
                       nrm2l, scll, nc.gpsimd)
            transpose_group(fl, [zlT[:, 0:SUB], zlT[:, SUB:2 * SUB]])

            fs4 = fsort.rearrange("(g t p) d -> g p t d", p=128, t=TILES_I)
            nrm2a = small.tile([128, GROUPS * TILES_I], FP32, tag="nrm2a")
            scla = small.tile([128, GROUPS * TILES_I], FP32, tag="scla")
            for g in range(GROUPS):
                fg = fgp.tile([128, TILES_I, D], FP32, tag=f"fg{g}")
                sl = ds(g * TILES_I, TILES_I)
                norm_group(fs4[g], fg, nrm2a[:, sl], scla[:, sl],
                           nc.sync if g % 2 == 0 else nc.gpsimd)
                transpose_group(fg, [ztc[2 * g][:], ztc[2 * g + 1][:]])

            # S = sum_j z_j ; S1 = sum_{j<n1} z_j  (sorted: 1s first)
            scols = small.tile([128, NSUB], FP32, tag="scols")
            for k in range(NSUB):
                nc.vector.reduce_sum(scols[:, k:k + 1], ztc[k][:], axis=AX.X)
            svec = small.tile([128, 2], FP32, tag="svec")
            s1p = small.tile([128, 2], FP32, tag="s1p")
            nc.vector.reduce_sum(svec[:, 0:1], scols[:], axis=AX.X)
            kf, w1 = n1 // SUB, n1 % SUB
            if kf > 0:
                nc.vector.reduce_sum(s1p[:, 0:1], scols[:, 0:kf], axis=AX.X)
            else:
                nc.vector.memset(s1p[:, 0:1], 0.0)
            if w1 > 0:
                nc.vector.reduce_sum(s1p[:, 1:2], ztc[kf][:, 0:w1], axis=AX.X)
            else:
                nc.vector.memset(s1p[:, 1:2], 0.0)
            nc.vector.reduce_sum(svec[:, 1:2], s1p[:], axis=AX.X)
            svecr = small.tile([128, 2], F32R, tag="svecr")
            nc.vector.tensor_copy(svecr[:], svec[:])

            # per-row-tile sim row sums: [z_i . S, z_i . S1]
            tall = small.tile([128, TILES_I], FP32, tag="tall")
            t1 = small.tile([128, TILES_I], FP32, tag="t1")
            for t in range(TILES_I):
                tp = psum.tile([128, CHUNK], FP32, tag="ps")
                nc.tensor.matmul(tp[:, 0:2], lhsT=zlT[:, ts(t, 128)],
                                 rhs=svecr[:], start=True, stop=True)
                nc.vector.tensor_copy(tall[:, t:t + 1], tp[:, 0:1])
                nc.vector.tensor_copy(t1[:, t:t + 1], tp[:, 1:2])

            # ---- main loop, chunk-major for setup/main overlap ----
            sacc = small.tile([128, TILES_I, nseg], FP32, tag="sacc")
            for q in range(NCHUNK):
                for t in range(TILES_I):
                    ps = psum.tile([128, CHUNK], FP32, tag="ps")
                    for s in range(CHUNK // SUB):
                        nc.tensor.matmul(
                            ps[:, ts(s, SUB)], lhsT=zlT[:, ts(t, 128)],
                            rhs=ztc[q * (CHUNK // SUB) + s][:],
                            start=True, stop=True)
                    for si, (lo, hi, _one) in enumerate(segs):
                        if lo >= q * CHUNK and hi <= (q + 1) * CHUNK:
                            nc.scalar.activation(
                                ps[:, lo - q * CHUNK:hi - q * CHUNK],
                                ps[:, lo - q * CHUNK:hi - q * CHUNK],
                                AF.Exp, scale=2.0,
                                accum_out=sacc[:, t, si:si + 1])

            # ---- finals ----
            simii = small.tile([128, TILES_I], FP32, tag="simii")
            eii = small.tile([128, TILES_I], FP32, tag="eii")
            nc.vector.tensor_mul(simii[:], nrm2l[:], scll[:])
            nc.vector.tensor_mul(simii[:], simii[:], scll[:])
            nc.scalar.activation(eii[:], simii[:], AF.Exp, scale=2.0)

            s1r = small.tile([128, TILES_I], FP32, tag="s1r")
            s0r = small.tile([128, TILES_I], FP32, tag="s0r")
            if cb > 0:
                nc.vector.reduce_sum(s1r[:], sacc[:, :, 0:cb], axis=AX.X)
            else:
                nc.vector.memset(s1r[:], 0.0)
            if cb < nseg:
                nc.vector.reduce_sum(s0r[:], sacc[:, :, cb:nseg], axis=AX.X)
            else:
                nc.vector.memset(s0r[:], 0.0)

            fin = small.tile([128, TILES_I], FP32, tag="fin")
            outs = small.tile([128, 3 * TILES_I], FP32, tag="outs")
            sall = small.tile([128, TILES_I], FP32, tag="sall")
            nc.vector.tensor_add(sall[:], s1r[:], s0r[:])
            # same = s0 + lab*(s1-s0)
            nc.vector.tensor_sub(fin[:], s1r[:], s0r[:])
            nc.vector.tensor_mul(fin[:], fin[:], lab_loc[:])
            nc.vector.tensor_add(fin[:], fin[:], s0r[:])
            num = small.tile([128, TILES_I], FP32, tag="num")
            nc.vector.tensor_sub(num[:], fin[:], eii[:])
            dennum = small.tile([128, TILES_I], FP32, tag="dennum")
            nc.vector.tensor_sub(dennum[:], sall[:], eii[:])
            nc.vector.tensor_scalar_add(out=num[:], in0=num[:], scalar1=EPS)
            # loss_row = ln(den+num) - ln(num+eps)
            lg1 = small.tile([128, TILES_I], FP32, tag="lg1")
            nc.scalar.activation(lg1[:], dennum[:], AF.Ln)
            nc.scalar.activation(outs[:, 0:TILES_I], num[:], AF.Ln)
            nc.vector.tensor_sub(outs[:, 0:TILES_I], lg1[:], outs[:, 0:TILES_I])

            # same_t = u + lab*(t1-u), u = tall - t1
            u = small.tile([128, TILES_I], FP32, tag="u")
            nc.vector.tensor_sub(u[:], tall[:], t1[:])
            nc.vector.tensor_sub(fin[:], t1[:], u[:])
            nc.vector.tensor_mul(fin[:], fin[:], lab_loc[:])
            nc.vector.tensor_add(fin[:], fin[:], u[:])
            nc.vector.tensor_sub(outs[:, TILES_I:2 * TILES_I], fin[:], simii[:])
            nc.vector.tensor_sub(outs[:, 2 * TILES_I:3 * TILES_I], tall[:], fin[:])

            nc.sync.dma_start(outp, outs[:])

    nc.compile()
    return nc


_NC_CACHE = {}


def _get_nc(n1: int = 4083):
    if n1 not in _NC_CACHE:
        _NC_CACHE[n1] = _build_kernel(n1)
    return _NC_CACHE[n1]


def prepare(feature: np.ndarray, label: np.ndarray):
    """Sort rows by label (1s first); build per-core input maps."""
    feature = np.ascontiguousarray(feature, dtype=np.float32)
    lab = np.asarray(label)
    perm = np.argsort(-lab, kind="stable")
    n1 = int((lab == 1).sum())
    fsort = np.ascontiguousarray(feature[perm])
    lsort = lab[perm].astype(np.float32)
    in_maps = []
    for c in range(N_CORES):
        sl = slice(c * LOCAL, (c + 1) * LOCAL)
        in_maps.append({
            "fsort": fsort,
            "floc": np.ascontiguousarray(fsort[sl]),
            "labloc": np.ascontiguousarray(
                lsort[sl].reshape(TILES_I, 128).T),
        })
    return n1, in_maps


def combine(results):
    P = np.stack([np.asarray(r["outp"], dtype=np.float64) for r in results])
    loss = P[:, :, 0:TILES_I].sum() / B
    mean_pos = P[:, :, TILES_I:2 * TILES_I].sum() / (float(B) * B)
    mean_neg = P[:, :, 2 * TILES_I:3 * TILES_I].sum() / (float(B) * B)
    return (np.float32(loss), np.float32(mean_pos), np.float32(mean_neg))


def run_on_hw(feature, label, **kwargs):
    n1, in_maps = prepare(feature, label)
    nc = _get_nc(n1)
    res = run_bass_kernel_spmd(nc, in_maps,
                               core_ids=list(range(N_CORES)), **kwargs)
    return combine(res.results), res


def kernel(feature: np.ndarray, label: np.ndarray):
    out, _ = run_on_hw(feature, label)
    return out



# revision 3
# speedup vs baseline: 3.3161x; 3.3161x over previous
"""ContrastiveLoss Trainium2 kernel (v2).

Contract: kernel(feature, label) -> (loss, mean_pos, mean_neg), matching
the reference. Full inputs in, full outputs out; internally sharded across
8 NeuronCores (each core owns 1024 rows of z and computes its [1024, 8192]
similarity slab).

Host prep (per the sharding hint, devices receive z): sort rows by label
(1s first), L2-normalize, pre-transpose z^T; bf16 copy for the sim matmuls,
fp32 local slice + [S, S1] column sums for the exact mean_pos/mean_neg path.

Device per core:
  - 64 bf16 matmuls (N=1024) build the 1024x8192 sim slab in PSUM chunks.
  - exp(2*sim) row-sums split between ScalarE (exact ACT exp + accum, first
    W_S cols of each 2048 chunk) and VectorE (Schraudolph bit-trick exp:
    one affine->int32 op + reduce over the fp32-bitcast, remaining cols).
    Label-sorted columns make the masked sums range splits.
  - 8 tiny fp32 matmuls z_loc @ [S, S1] give the mean accumulators.
Host combine: logs for the per-row loss (shipped as num/den sums), scalar
reductions across cores.
"""

import sys

sys.path.insert(0, "/opt/trn_rl_repo")

import numpy as np
import ml_dtypes

import concourse.bass as bass
import concourse.mybir as mybir
import concourse.tile as tile
from concourse import bacc
from concourse.bass import ds, ts
from concourse.bass_utils import run_bass_kernel_spmd

B = 8192
D = 128
N_CORES = 8
LOCAL = B // N_CORES          # 1024 rows per core
TILES_I = LOCAL // 128        # 8 local row tiles per core
CHUNK = 2048                  # psum chunk width (4 banks)
NCHUNK = B // CHUNK
W_S = 1152                    # scalar-engine exp columns per chunk
EPS = 1e-8
EII = float(np.exp(2.0))      # exp(2*sim_ii), sim_ii == 1 after normalize

LOG2E = 1.4426950408889634
SCHRA_A = float(np.float32(2.0 * LOG2E * (1 << 23)))
SCHRA_C = 298765
SCHRA_B = float(np.float32((127 << 23) - SCHRA_C))

FP32 = mybir.dt.float32
BF16 = mybir.dt.bfloat16
INT32 = mybir.dt.int32
AF = mybir.ActivationFunctionType
ALU = mybir.AluOpType
AX = mybir.AxisListType


def _segments(n1):
    """Per-chunk exp segments: (lo, hi, is_label1, engine) with engine
    's' for ScalarE (first W_S cols of each chunk) or 'v' for VectorE."""
    segs = []
    for q in range(NCHUNK):
        for lo, hi, eng in ((q * CHUNK, q * CHUNK + W_S, 's'),
                            (q * CHUNK + W_S, (q + 1) * CHUNK, 'v')):
            if n1 <= lo:
                segs.append((lo, hi, False, eng))
            elif n1 >= hi:
                segs.append((lo, hi, True, eng))
            else:
                segs.append((lo, n1, True, eng))
                segs.append((n1, hi, False, eng))
    # label-1 segments first so s1/s0 are contiguous reduces over sacc
    segs = [s for s in segs if s[2]] + [s for s in segs if not s[2]]
    return segs


def _build_kernel(n1: int):
    nc = bacc.Bacc("TRN2", target_bir_lowering=False, debug=False,
                   num_devices=N_CORES)
    ztb = nc.dram_tensor("ztb", [128, B], BF16, kind="ExternalInput").ap()
    zltb = nc.dram_tensor("zltb", [128, LOCAL], BF16,
                          kind="ExternalInput").ap()
    zl = nc.dram_tensor("zl", [128, LOCAL], FP32, kind="ExternalInput").ap()
    svec = nc.dram_tensor("svec", [128, 2], FP32, kind="ExternalInput").ap()
    labloc = nc.dram_tensor("labloc", [128, TILES_I], FP32,
                            kind="ExternalInput").ap()
    outp = nc.dram_tensor("outp", [128, 4 * TILES_I], FP32,
                          kind="ExternalOutput").ap()

    segs = _segments(n1)
    nseg = len(segs)
    cb = sum(1 for s in segs if s[2])
    W_V = CHUNK - W_S

    with tile.TileContext(nc) as tc:
        with (
            tc.tile_pool(name="zp", bufs=1) as zp,
            tc.tile_pool(name="small", bufs=1) as small,
            tc.tile_pool(name="scr", bufs=2) as scrp,
            tc.tile_pool(name="psum", bufs=2, space=bass.MemorySpace.PSUM) as psum,
        ):
            # fp32 local z^T + svec + labels on the sync queue (needed first)
            zlt = small.tile([128, LOCAL], FP32, tag="zlt")
            nc.sync.dma_start(zlt[:], zl)
            sv = small.tile([128, 2], FP32, tag="sv")
            nc.sync.dma_start(sv[:], svec)
            lab = small.tile([128, TILES_I], FP32, tag="lab")
            nc.sync.dma_start(lab[:], labloc)
            zlb = small.tile([128, LOCAL], BF16, tag="zlb")
            nc.sync.dma_start(zlb[:], zltb)

            # big bf16 z^T, chunked so chunk 0 compute can start early
            zt = zp.tile([128, B], BF16, tag="zt")
            for q in range(NCHUNK):
                nc.gpsimd.dma_start(zt[:, ts(q, CHUNK)], ztb[:, ts(q, CHUNK)])

            # tall/t1 = z_loc @ [S, S1]  (fp32, exact path for the means)
            pt = psum.tile([128, CHUNK], FP32, tag="ps")
            for t in range(TILES_I):
                nc.tensor.matmul(pt[:, 2 * t:2 * t + 2],
                                 lhsT=zlt[:, ts(t, 128)], rhs=sv[:],
                                 start=True, stop=True)
            tt = small.tile([128, 2 * TILES_I], FP32, tag="tt")
            nc.vector.tensor_copy(tt[:], pt[:, 0:2 * TILES_I])

            # ---- main loop: sim chunks + split exp row sums ----
            sacc = small.tile([128, TILES_I, nseg], FP32, tag="sacc")
            for q in range(NCHUNK):
                qsegs = [(i, s) for i, s in enumerate(segs)
                         if q * CHUNK <= s[0] and s[1] <= (q + 1) * CHUNK]
                for t in range(TILES_I):
                    ps = psum.tile([128, CHUNK], FP32, tag="ps")
                    for h in range(4):
                        nc.tensor.matmul(
                            ps[:, ts(h, 512)], lhsT=zlb[:, ts(t, 128)],
                            rhs=zt[:, q * CHUNK + h * 512:
                                   q * CHUNK + (h + 1) * 512],
                            start=True, stop=True)
                    scr = scrp.tile([128, W_V], INT32, tag="scr")
                    scrf = scr.bitcast(FP32)
                    for si, (lo, hi, _l1, eng) in qsegs:
                        if eng == 's':
                            nc.scalar.activation(
                                ps[:, lo - q * CHUNK:hi - q * CHUNK],
                                ps[:, lo - q * CHUNK:hi - q * CHUNK],
                                AF.Exp, scale=2.0,
                                accum_out=sacc[:, t, si:si + 1])
                    # one affine->int32 pass for the whole DVE range
                    nc.vector.tensor_scalar(
                        out=scr[:], in0=ps[:, W_S:CHUNK],
                        scalar1=SCHRA_A, scalar2=SCHRA_B,
                        op0=ALU.mult, op1=ALU.add)
                    for si, (lo, hi, _l1, eng) in qsegs:
                        if eng == 'v':
                            o = q * CHUNK + W_S
                            nc.vector.reduce_sum(
                                sacc[:, t, si:si + 1],
                                scrf[:, lo - o:hi - o], axis=AX.X)

            # ---- finals ----
            s1r = small.tile([128, TILES_I], FP32, tag="s1r")
            s0r = small.tile([128, TILES_I], FP32, tag="s0r")
            if cb > 0:
                nc.vector.reduce_sum(s1r[:], sacc[:, :, 0:cb], axis=AX.X)
            else:
                nc.vector.memset(s1r[:], 0.0)
            if cb < nseg:
                nc.vector.reduce_sum(s0r[:], sacc[:, :, cb:nseg], axis=AX.X)
            else:
                nc.vector.memset(s0r[:], 0.0)

            outs = small.tile([128, 4 * TILES_I], FP32, tag="outs")
            sall = small.tile([128, TILES_I], FP32, tag="sall")
            fin = small.tile([128, TILES_I], FP32, tag="fin")
            nc.vector.tensor_add(sall[:], s1r[:], s0r[:])
            # same = s0 + lab*(s1-s0)
            nc.vector.tensor_sub(fin[:], s1r[:], s0r[:])
            nc.vector.tensor_mul(fin[:], fin[:], lab[:])
            nc.vector.tensor_add(fin[:], fin[:], s0r[:])
            nc.vector.tensor_scalar_add(out=outs[:, 0:TILES_I], in0=fin[:],
                                        scalar1=-EII)
            nc.vector.tensor_scalar_add(out=outs[:, TILES_I:2 * TILES_I],
                                        in0=sall[:], scalar1=-EII)

            # means partials from tall/t1 (tt cols interleaved [tall, t1])
            tall = tt[:, 0:2 * TILES_I:2]
            t1 = tt[:, 1:2 * TILES_I:2]
            u = small.tile([128, TILES_I], FP32, tag="u")
            nc.vector.tensor_sub(u[:], tall, t1)
            # same_t = u + lab*(t1-u)
            nc.vector.tensor_sub(fin[:], t1, u[:])
            nc.vector.tensor_mul(fin[:], fin[:], lab[:])
            nc.vector.tensor_add(fin[:], fin[:], u[:])
            # pos = same_t - sim_ii (= 1), neg = tall - same_t
            nc.vector.tensor_scalar_add(out=outs[:, 2 * TILES_I:3 * TILES_I],
                                        in0=fin[:], scalar1=-1.0)
            nc.vector.tensor_sub(outs[:, 3 * TILES_I:4 * TILES_I], tall,
                                 fin[:])

            nc.sync.dma_start(outp, outs[:])

    nc.compile()
    return nc


_NC_CACHE = {}


def _get_nc(n1: int = 4083):
    if n1 not in _NC_CACHE:
        _NC_CACHE[n1] = _build_kernel(n1)
    return _NC_CACHE[n1]


def prepare(feature: np.ndarray, label: np.ndarray):
    """Sort rows by label (1s first), normalize, transpose; per-core maps."""
    feature = np.ascontiguousarray(feature, dtype=np.float32)
    lab = np.asarray(label)
    perm = np.argsort(-lab, kind="stable")
    n1 = int((lab == 1).sum())
    f = feature[perm]
    nrm = np.maximum(np.sqrt((f.astype(np.float64) ** 2).sum(1)), 1e-12)
    z = (f / nrm[:, None]).astype(np.float32)
    zT = np.ascontiguousarray(z.T)                    # [128, B] fp32
    ztb = zT.astype(ml_dtypes.bfloat16)               # [128, B] bf16
    zf = z.astype(np.float64)
    S = zf.sum(0)
    S1 = zf[:n1].sum(0)
    sv = np.ascontiguousarray(
        np.stack([S, S1], axis=1).astype(np.float32))  # [128, 2]
    lsort = lab[perm].astype(np.float32)
    in_maps = []
    for c in range(N_CORES):
        sl = slice(c * LOCAL, (c + 1) * LOCAL)
        in_maps.append({
            "ztb": ztb,
            "zltb": np.ascontiguousarray(ztb[:, sl]),
            "zl": np.ascontiguousarray(zT[:, sl]),
            "svec": sv,
            "labloc": np.ascontiguousarray(
                lsort[sl].reshape(TILES_I, 128).T),
        })
    return n1, in_maps


def combine(results):
    P = np.stack([np.asarray(r["outp"], dtype=np.float64) for r in results])
    num = P[:, :, 0:TILES_I]
    dennum = P[:, :, TILES_I:2 * TILES_I]
    loss = (np.log(dennum) - np.log(np.maximum(num, 0.0) + EPS)).sum() / B
    mean_pos = P[:, :, 2 * TILES_I:3 * TILES_I].sum() / (float(B) * B)
    mean_neg = P[:, :, 3 * TILES_I:4 * TILES_I].sum() / (float(B) * B)
    return (np.float32(loss), np.float32(mean_pos), np.float32(mean_neg))


def run_on_hw(feature, label, **kwargs):
    n1, in_maps = prepare(feature, label)
    nc = _get_nc(n1)
    res = run_bass_kernel_spmd(nc, in_maps,
                               core_ids=list(range(N_CORES)), **kwargs)
    return combine(res.results), res


def kernel(feature: np.ndarray, label: np.ndarray):
    out, _ = run_on_hw(feature, label)
    return out
